# revision 15
# baseline (speedup 1.0000x reference)
"""MoE routing kernel for trn2 (8 NeuronCores, expert-parallel).

Computes the dense-MoE reference:
    logits = x @ router_w; p = softmax(logits); top2 renormalized weights
    out = sum_e we[t,e] * (silu(x@w1[e]) * (x@v1[e])) @ w2[e]

Sharding: expert-parallel — core r holds expert r's weights, all tokens.
Each core computes its expert's weighted partial output out_e^T [D, T],
then a ReduceScatter over the 8 cores sums partials; core r keeps D-rows
[r*D/8, (r+1)*D/8). Host concatenates the shards and transposes.

Router is replicated on every core; the per-core expert weight column is
  we[t] = (l_e >= m2) * sigmoid(2*l_e - m1 - m2)
where m1/m2 are the top-2 logit values — exactly the renormalized top-2
softmax weight (full-softmax denominator cancels).

All matmuls run in float32r (fp32 data, 1 cycle/row on the PE vs 4 for
plain fp32; ~1.5e-4 matmul rel err measured on hw).
"""

import os

import numpy as np

import concourse.bass as bass
import concourse.mybir as mybir
import concourse.tile as tile
from concourse import bacc
from concourse.bass_utils import run_bass_kernel_spmd
from concourse.masks import make_identity

P = 128
N_CORES = 8
F32 = mybir.dt.float32
F32R = mybir.dt.float32r
AX = mybir.AxisListType
ALU = mybir.AluOpType
ACTF = mybir.ActivationFunctionType
BIG = 1.0e9


def _install_trace_hook_if_requested():
    """Optional: enables NTFF profiling when BASS_TRACE=1 (dev only)."""
    if os.environ.get("BASS_TRACE") != "1":
        return
    import sys
    import types

    if "antenv.axon_hooks" in sys.modules:
        return
    mod = types.ModuleType("antenv.axon_hooks")
    state = {"hook": None}
    mod.set_axon_ntff_profile_hook = lambda h: state.__setitem__("hook", h)
    mod.get_axon_ntff_profile_hook = lambda: state["hook"]
    sys.modules["antenv.axon_hooks"] = mod
    try:
        from trn_agent_boot.trn_boot import _ntff_profile_via_ctypes

        mod.set_axon_ntff_profile_hook(
            _ntff_profile_via_ctypes("/opt/axon/libaxon_pjrt.so")
        )
    except Exception:
        pass


def build(T, D, F, E, t_chunk):
    """Build the SPMD per-core bass program (see module docstring)."""
    assert T % t_chunk == 0 and t_chunk % P == 0 and t_chunk <= 512
    assert D % P == 0 and F % P == 0
    DC = D // P          # contraction chunks over D
    FT = F // P          # f tiles (partition tiles of F)
    DT = D // P          # output d tiles
    TC = T // t_chunk    # token chunks
    NT = t_chunk // P    # token tiles per chunk
    DS = D // N_CORES    # output shard rows per core
    assert DT % 2 == 0

    nc = bacc.Bacc("TRN2", target_bir_lowering=False, debug=False,
                   num_devices=N_CORES)

    xT = nc.dram_tensor("xT", [D, T], F32R, kind="ExternalInput")
    w1 = nc.dram_tensor("w1", [D, F], F32R, kind="ExternalInput")
    v1 = nc.dram_tensor("v1", [D, F], F32R, kind="ExternalInput")
    w2 = nc.dram_tensor("w2", [F, D], F32R, kind="ExternalInput")
    rw = nc.dram_tensor("rw", [D, E], F32, kind="ExternalInput")
    eoh = nc.dram_tensor("eoh", [P, E], F32, kind="ExternalInput")
    out_shards = nc.dram_tensor("out_shards", [TC, DS, t_chunk], F32,
                                kind="ExternalOutput")

    with tile.TileContext(nc) as tc:
        with (
            tc.tile_pool(name="const", bufs=1) as const,
            tc.tile_pool(name="xpool", bufs=DC + 1) as xpool,
            tc.tile_pool(name="wpool", bufs=3) as wpool,
            tc.tile_pool(name="w2pool", bufs=2) as w2pool,
            tc.tile_pool(name="gpool", bufs=FT + 1) as gpool,
            tc.tile_pool(name="rpool", bufs=2) as rpool,
            tc.tile_pool(name="opool", bufs=2) as opool,
            tc.tile_pool(name="pmain", bufs=2, space="PSUM") as pmain,
            tc.tile_pool(name="paux", bufs=2, space="PSUM") as paux,
            tc.tile_pool(name="dram", bufs=2, space="DRAM") as dram,
            tc.tile_pool(name="dramsh", bufs=4, space="DRAM") as dramsh,
        ):
            ones = const.tile([1, P], F32)
            nc.vector.memset(ones[:], 1.0)
            ident = const.tile([P, P], F32)
            make_identity(nc, ident)
            eoh_sb = const.tile([P, E], F32)
            nc.sync.dma_start(eoh_sb[:], eoh[:])
            rw_sb = const.tile([P, DC, E], F32)
            nc.sync.dma_start(rw_sb[:], rw.rearrange("(i p) e -> p i e", p=P))

            def load_x(c):
                tiles = []
                for d in range(DC):
                    xt_d = xpool.tile([P, t_chunk], F32R, name="x_sb")
                    nc.sync.dma_start(
                        xt_d[:],
                        xT[d * P:(d + 1) * P,
                           c * t_chunk:(c + 1) * t_chunk],
                    )
                    tiles.append(xt_d)
                return tiles

            def router(c_tok):
                """Exact-fp32 router -> per-token expert weight we_sb [P, NT].

                logitsT [E, t_chunk] = rw.T @ x (fp32, rw stationary), then
                PE-transposed per 128-token tile into [P, E] for the free-axis
                top-2 math. fp32 keeps top-2 selection bit-faithful."""
                # logitsT [E, t_chunk] in exact fp32 (rw stationary, so
                # weight loads are trivial); x re-read as true fp32 bytes.
                ps_lt = paux.tile([P, t_chunk], F32, name="ps_aux")[:E, :]
                for d in range(DC):
                    x32_d = rpool.tile([P, t_chunk], F32, name="x32")
                    nc.scalar.dma_start(
                        x32_d[:],
                        xT[d * P:(d + 1) * P,
                           c_tok:c_tok + t_chunk].bitcast(F32),
                    )
                    nc.tensor.matmul(ps_lt[:], rw_sb[:, d, :], x32_d[:],
                                     start=(d == 0), stop=(d == DC - 1))
                ltT = rpool.tile([E, t_chunk], F32, name="ltT")
                nc.vector.tensor_copy(ltT[:], ps_lt[:])
                we_sb = rpool.tile([P, NT], F32, name="we_sb")
                for j in range(NT):
                    ps_lg = paux.tile([P, t_chunk], F32,
                                      name="ps_aux")[:, :E]
                    nc.tensor.transpose(ps_lg[:],
                                        ltT[:, j * P:(j + 1) * P],
                                        ident[:E, :E])
                    lg = rpool.tile([P, E], F32, name="lg")
                    nc.vector.tensor_copy(lg[:], ps_lg[:])
                    m1 = rpool.tile([P, 1], F32, name="m1")
                    nc.vector.reduce_max(m1[:], lg[:], axis=AX.X)
                    mk = rpool.tile([P, E], F32, name="mk")
                    nc.vector.tensor_scalar(mk[:], lg[:], m1[:], BIG,
                                            op0=ALU.is_ge, op1=ALU.mult)
                    msk = rpool.tile([P, E], F32, name="msk")
                    nc.vector.tensor_sub(msk[:], lg[:], mk[:])
                    m2 = rpool.tile([P, 1], F32, name="m2")
                    nc.vector.reduce_max(m2[:], msk[:], axis=AX.X)
                    nb = rpool.tile([P, 1], F32, name="nb")
                    nc.vector.tensor_scalar(nb[:], m1[:], m2[:], -1.0,
                                            op0=ALU.add, op1=ALU.mult)
                    sg = rpool.tile([P, E], F32, name="sg")
                    nc.scalar.activation(sg[:], lg[:], ACTF.Sigmoid,
                                         bias=nb[:], scale=2.0)
                    keep = rpool.tile([P, E], F32, name="keep")
                    nc.vector.tensor_scalar(keep[:], lg[:], m2[:], None,
                                            op0=ALU.is_ge)
                    wsel = rpool.tile([P, E], F32, name="wsel")
                    nc.vector.tensor_mul(wsel[:], sg[:], keep[:])
                    nc.vector.tensor_mul(wsel[:], wsel[:], eoh_sb[:])
                    nc.vector.reduce_sum(we_sb[:, j:j + 1], wsel[:],
                                         axis=AX.X)
                return we_sb

            def build_we_bc(we_sb):
                """we_sb [P(token), NT] -> we_bc [P, t_chunk] broadcast along
                partitions (token index on the free axis), PE-only."""
                ps_t = paux.tile([P, t_chunk], F32, name="ps_aux")[:1, :]
                for j in range(NT):
                    nc.tensor.transpose(ps_t[:, j * P:(j + 1) * P],
                                        we_sb[:, j:j + 1], ident[:])
                werow = rpool.tile([1, t_chunk], F32, name="werow")
                nc.vector.tensor_copy(werow[:], ps_t[:])
                ps_b = paux.tile([P, t_chunk], F32, name="ps_aux")
                nc.tensor.matmul(ps_b[:], ones[:], werow[:],
                                 start=True, stop=True)
                we_bc = rpool.tile([P, t_chunk], F32, name="we_bc")
                nc.vector.tensor_copy(we_bc[:], ps_b[:])
                return we_bc

            def load_w2(dt):
                w2_cb = w2pool.tile([P, FT, P], F32R, name="w2_cb")
                eng = nc.sync if dt % 2 == 0 else nc.gpsimd
                eng.dma_start(
                    w2_cb[:],
                    w2[:, dt * P:(dt + 1) * P].rearrange(
                        "(i p) d -> p i d", p=P),
                )
                return w2_cb

            # ---------------- software-pipelined chunk loop ----------------
            x_tiles = load_x(0)
            we_sb = router(0)

            for c in range(TC):
                # phase 1: gT[f] = silu(w1.T x) * (v1.T x)   [f32r]
                gts = []
                for f in range(FT):
                    w1_cb = wpool.tile([P, DC, P], F32R, name="w1_cb")
                    nc.sync.dma_start(
                        w1_cb[:],
                        w1[:, f * P:(f + 1) * P].rearrange(
                            "(i p) f -> p i f", p=P),
                    )
                    v1_cb = wpool.tile([P, DC, P], F32R, name="v1_cb")
                    nc.gpsimd.dma_start(
                        v1_cb[:],
                        v1[:, f * P:(f + 1) * P].rearrange(
                            "(i p) f -> p i f", p=P),
                    )
                    ps_h = pmain.tile([P, t_chunk], F32, name="ps_h")
                    for d in range(DC):
                        nc.tensor.matmul(ps_h[:], w1_cb[:, d, :],
                                         x_tiles[d][:],
                                         start=(d == 0), stop=(d == DC - 1))
                    ps_v = pmain.tile([P, t_chunk], F32, name="ps_v")
                    for d in range(DC):
                        nc.tensor.matmul(ps_v[:], v1_cb[:, d, :],
                                         x_tiles[d][:],
                                         start=(d == 0), stop=(d == DC - 1))
                    sl = opool.tile([P, t_chunk], F32, name="sl")
                    nc.scalar.activation(sl[:], ps_h[:], ACTF.Silu)
                    gt = gpool.tile([P, t_chunk], F32R, name="gt")
                    nc.vector.tensor_mul(gt[:], sl[:], ps_v[:])
                    gts.append(gt)

                # expert-weight broadcast for this chunk (inputs long ready)
                we_bc = build_we_bc(we_sb)

                # prefetch first w2 blocks, then next chunk's activations
                w2_pre = [load_w2(0), load_w2(1)]
                if c + 1 < TC:
                    nx_tiles = load_x(c + 1)

                # phase 2: outT[dt] = (sum_f w2[f,dt].T gT[f]) * we
                rs_halves = []
                for half in range(2):
                    rs_in = dram.tile([D // 2, t_chunk], F32, name="rs_in")
                    for k in range(DT // 2):
                        dt = half * (DT // 2) + k
                        w2_cb = w2_pre[dt] if dt < len(w2_pre) \
                            else load_w2(dt)
                        ps_o = pmain.tile([P, t_chunk], F32, name="ps_o")
                        for f in range(FT):
                            nc.tensor.matmul(ps_o[:], w2_cb[:, f, :],
                                             gts[f][:],
                                             start=(f == 0),
                                             stop=(f == FT - 1))
                        ob = opool.tile([P, t_chunk], F32, name="ob")
                        nc.vector.tensor_mul(ob[:], ps_o[:], we_bc[:])
                        nc.scalar.dma_start(rs_in[k * P:(k + 1) * P, :],
                                            ob[:])
                    rs_out = dramsh.tile([D // 2 // N_CORES, t_chunk], F32,
                                         name="rs_out")
                    nc.gpsimd.collective_compute(
                        "ReduceScatter",
                        ALU.add,
                        replica_groups=[list(range(N_CORES))],
                        ins=[rs_in[:].opt()],
                        outs=[rs_out[:].opt()],
                    )
                    rs_halves.append(rs_out)

                # next chunk's router (x loaded during phase 2 above)
                if c + 1 < TC:
                    x_tiles = nx_tiles
                    we_sb = router((c + 1) * t_chunk)

                # ship this chunk's shards (waits on RS via tile deps)
                HS = D // 2 // N_CORES
                for half, rs_out in enumerate(rs_halves):
                    nc.sync.dma_start(
                        out_shards[c, half * HS:(half + 1) * HS, :],
                        rs_out[:])

    nc.finalize()
    return nc


_CACHE = {}
LAST_RESULTS = None


def _get_nc(T, D, F, E, t_chunk):
    key = (T, D, F, E, t_chunk)
    if key not in _CACHE:
        _CACHE[key] = build(*key)
    return _CACHE[key]


def run_moe(hidden_states, router_w, w1, v1, w2, t_chunk=512):
    global LAST_RESULTS
    _install_trace_hook_if_requested()

    B, S, D = hidden_states.shape
    E = router_w.shape[1]
    F = w1.shape[2]
    T = B * S
    DS = D // N_CORES
    TCN = T // t_chunk

    x = np.ascontiguousarray(hidden_states.reshape(T, D).astype(np.float32))
    xT = np.ascontiguousarray(x.T)
    rwc = np.ascontiguousarray(router_w.astype(np.float32))

    nc = _get_nc(T, D, F, E, t_chunk)

    in_maps = []
    for r in range(N_CORES):
        ohr = np.zeros((P, E), dtype=np.float32)
        ohr[:, r] = 1.0
        in_maps.append({
            "xT": xT,
            "w1": np.ascontiguousarray(w1[r].astype(np.float32)),
            "v1": np.ascontiguousarray(v1[r].astype(np.float32)),
            "w2": np.ascontiguousarray(w2[r].astype(np.float32)),
            "rw": rwc,
            "eoh": ohr,
        })

    res = run_bass_kernel_spmd(nc, in_maps, core_ids=list(range(N_CORES)))
    LAST_RESULTS = res

    HS = D // 2 // N_CORES
    fullT = np.empty((D, T), dtype=np.float32)
    for r in range(N_CORES):
        sh = res.results[r]["out_shards"]  # [TCN, DS, t_chunk]
        for c in range(TCN):
            cols = slice(c * t_chunk, (c + 1) * t_chunk)
            for h in range(2):
                fullT[h * (D // 2) + r * HS:
                      h * (D // 2) + (r + 1) * HS, cols] = \
                    sh[c, h * HS:(h + 1) * HS]
    return np.ascontiguousarray(fullT.T).reshape(B, S, D)


def kernel(hidden_states, router_w, w1, v1, w2):
    return run_moe(hidden_states, router_w, w1, v1, w2, t_chunk=512)


# revision 16
# speedup vs baseline: 1.0181x; 1.0181x over previous
"""MoE routing kernel for trn2 (8 NeuronCores, expert-parallel).

Computes the dense-MoE reference:
    logits = x @ router_w; p = softmax(logits); top2 renormalized weights
    out = sum_e we[t,e] * (silu(x@w1[e]) * (x@v1[e])) @ w2[e]

Sharding: expert-parallel — core r holds expert r's weights, all tokens.
Each core computes its expert's weighted partial output out_e^T [D, T],
then a ReduceScatter over the 8 cores sums partials; core r keeps D-rows
[r*D/8, (r+1)*D/8). Host concatenates the shards and transposes.

Router is replicated on every core; the per-core expert weight column is
  we[t] = (l_e >= m2) * sigmoid(2*l_e - m1 - m2)
where m1/m2 are the top-2 logit values — exactly the renormalized top-2
softmax weight (full-softmax denominator cancels).

All matmuls run in float32r (fp32 data, 1 cycle/row on the PE vs 4 for
plain fp32; ~1.5e-4 matmul rel err measured on hw).
"""

import os

import numpy as np

import concourse.bass as bass
import concourse.mybir as mybir
import concourse.tile as tile
from concourse import bacc
from concourse.bass_utils import run_bass_kernel_spmd
from concourse.masks import make_identity

P = 128
N_CORES = 8
F32 = mybir.dt.float32
F32R = mybir.dt.float32r
AX = mybir.AxisListType
ALU = mybir.AluOpType
ACTF = mybir.ActivationFunctionType
BIG = 1.0e9


def _install_trace_hook_if_requested():
    """Optional: enables NTFF profiling when BASS_TRACE=1 (dev only)."""
    if os.environ.get("BASS_TRACE") != "1":
        return
    import sys
    import types

    if "antenv.axon_hooks" in sys.modules:
        return
    mod = types.ModuleType("antenv.axon_hooks")
    state = {"hook": None}
    mod.set_axon_ntff_profile_hook = lambda h: state.__setitem__("hook", h)
    mod.get_axon_ntff_profile_hook = lambda: state["hook"]
    sys.modules["antenv.axon_hooks"] = mod
    try:
        from trn_agent_boot.trn_boot import _ntff_profile_via_ctypes

        mod.set_axon_ntff_profile_hook(
            _ntff_profile_via_ctypes("/opt/axon/libaxon_pjrt.so")
        )
    except Exception:
        pass


def build(T, D, F, E, t_chunk):
    """Build the SPMD per-core bass program (see module docstring)."""
    assert T % t_chunk == 0 and t_chunk % P == 0 and t_chunk <= 512
    assert D % P == 0 and F % P == 0
    DC = D // P          # contraction chunks over D
    FT = F // P          # f tiles (partition tiles of F)
    DT = D // P          # output d tiles
    TC = T // t_chunk    # token chunks
    NT = t_chunk // P    # token tiles per chunk
    DS = D // N_CORES    # output shard rows per core
    N_RS = 4 if DT % 4 == 0 else 2
    assert DT % N_RS == 0

    nc = bacc.Bacc("TRN2", target_bir_lowering=False, debug=False,
                   num_devices=N_CORES)

    xT = nc.dram_tensor("xT", [D, T], F32R, kind="ExternalInput")
    w1 = nc.dram_tensor("w1", [D, F], F32R, kind="ExternalInput")
    v1 = nc.dram_tensor("v1", [D, F], F32R, kind="ExternalInput")
    w2 = nc.dram_tensor("w2", [F, D], F32R, kind="ExternalInput")
    rw = nc.dram_tensor("rw", [D, E], F32, kind="ExternalInput")
    eoh = nc.dram_tensor("eoh", [P, E], F32, kind="ExternalInput")
    out_shards = nc.dram_tensor("out_shards", [TC, DS, t_chunk], F32,
                                kind="ExternalOutput")

    with tile.TileContext(nc) as tc:
        with (
            tc.tile_pool(name="const", bufs=1) as const,
            tc.tile_pool(name="xpool", bufs=DC + 1) as xpool,
            tc.tile_pool(name="wpool", bufs=3) as wpool,
            tc.tile_pool(name="w2pool", bufs=2) as w2pool,
            tc.tile_pool(name="gpool", bufs=FT) as gpool,
            tc.tile_pool(name="rpool", bufs=2) as rpool,
            tc.tile_pool(name="x32pool", bufs=3) as x32pool,
            tc.tile_pool(name="opool", bufs=2) as opool,
            tc.tile_pool(name="pmain", bufs=2, space="PSUM") as pmain,
            tc.tile_pool(name="paux", bufs=2, space="PSUM") as paux,
            tc.tile_pool(name="dram", bufs=3, space="DRAM") as dram,
            tc.tile_pool(name="dramsh", bufs=6, space="DRAM") as dramsh,
        ):
            ones = const.tile([1, P], F32)
            nc.vector.memset(ones[:], 1.0)
            ident = const.tile([P, P], F32)
            make_identity(nc, ident)
            eoh_sb = const.tile([P, E], F32)
            nc.sync.dma_start(eoh_sb[:], eoh[:])
            rw_sb = const.tile([P, DC, E], F32)
            nc.sync.dma_start(rw_sb[:], rw.rearrange("(i p) e -> p i e", p=P))

            def load_x(c):
                tiles = []
                for d in range(DC):
                    xt_d = xpool.tile([P, t_chunk], F32R, name="x_sb")
                    nc.scalar.dma_start(
                        xt_d[:],
                        xT[d * P:(d + 1) * P,
                           c * t_chunk:(c + 1) * t_chunk],
                    )
                    tiles.append(xt_d)
                return tiles

            def router(c_tok):
                """Exact-fp32 router -> per-token expert weight we_sb [P, NT].

                logitsT [E, t_chunk] = rw.T @ x (fp32, rw stationary), then
                PE-transposed per 128-token tile into [P, E] for the free-axis
                top-2 math. fp32 keeps top-2 selection bit-faithful."""
                # logitsT [E, t_chunk] in exact fp32 (rw stationary, so
                # weight loads are trivial); x re-read as true fp32 bytes.
                ps_lt = paux.tile([P, t_chunk], F32, name="ps_aux")[:E, :]
                for d in range(DC):
                    x32_d = x32pool.tile([P, t_chunk], F32, name="x32")
                    nc.scalar.dma_start(
                        x32_d[:],
                        xT[d * P:(d + 1) * P,
                           c_tok:c_tok + t_chunk].bitcast(F32),
                    )
                    nc.tensor.matmul(ps_lt[:], rw_sb[:, d, :], x32_d[:],
                                     start=(d == 0), stop=(d == DC - 1))
                ltT = rpool.tile([E, t_chunk], F32, name="ltT")
                nc.vector.tensor_copy(ltT[:], ps_lt[:])
                we_sb = rpool.tile([P, NT], F32, name="we_sb")
                for j in range(NT):
                    ps_lg = paux.tile([P, t_chunk], F32,
                                      name="ps_aux")[:, :E]
                    nc.tensor.transpose(ps_lg[:],
                                        ltT[:, j * P:(j + 1) * P],
                                        ident[:E, :E])
                    lg = rpool.tile([P, E], F32, name="lg")
                    nc.vector.tensor_copy(lg[:], ps_lg[:])
                    m1 = rpool.tile([P, 1], F32, name="m1")
                    nc.vector.reduce_max(m1[:], lg[:], axis=AX.X)
                    mk = rpool.tile([P, E], F32, name="mk")
                    nc.vector.tensor_scalar(mk[:], lg[:], m1[:], BIG,
                                            op0=ALU.is_ge, op1=ALU.mult)
                    msk = rpool.tile([P, E], F32, name="msk")
                    nc.vector.tensor_sub(msk[:], lg[:], mk[:])
                    m2 = rpool.tile([P, 1], F32, name="m2")
                    nc.vector.reduce_max(m2[:], msk[:], axis=AX.X)
                    nb = rpool.tile([P, 1], F32, name="nb")
                    nc.vector.tensor_scalar(nb[:], m1[:], m2[:], -1.0,
                                            op0=ALU.add, op1=ALU.mult)
                    sg = rpool.tile([P, E], F32, name="sg")
                    nc.scalar.activation(sg[:], lg[:], ACTF.Sigmoid,
                                         bias=nb[:], scale=2.0)
                    keep = rpool.tile([P, E], F32, name="keep")
                    nc.vector.tensor_scalar(keep[:], lg[:], m2[:], None,
                                            op0=ALU.is_ge)
                    wsel = rpool.tile([P, E], F32, name="wsel")
                    nc.vector.tensor_mul(wsel[:], sg[:], keep[:])
                    nc.vector.tensor_mul(wsel[:], wsel[:], eoh_sb[:])
                    nc.vector.reduce_sum(we_sb[:, j:j + 1], wsel[:],
                                         axis=AX.X)
                return we_sb

            def build_we_bc(we_sb):
                """we_sb [P(token), NT] -> we_bc [P, t_chunk] broadcast along
                partitions (token index on the free axis), PE-only."""
                ps_t = paux.tile([P, t_chunk], F32, name="ps_aux")[:1, :]
                for j in range(NT):
                    nc.tensor.transpose(ps_t[:, j * P:(j + 1) * P],
                                        we_sb[:, j:j + 1], ident[:])
                werow = rpool.tile([1, t_chunk], F32, name="werow")
                nc.vector.tensor_copy(werow[:], ps_t[:])
                ps_b = paux.tile([P, t_chunk], F32, name="ps_aux")
                nc.tensor.matmul(ps_b[:], ones[:], werow[:],
                                 start=True, stop=True)
                we_bc = rpool.tile([P, t_chunk], F32, name="we_bc")
                nc.vector.tensor_copy(we_bc[:], ps_b[:])
                return we_bc

            def load_w2(dt):
                w2_cb = w2pool.tile([P, FT, P], F32R, name="w2_cb")
                eng = nc.sync if dt % 2 == 0 else nc.gpsimd
                eng.dma_start(
                    w2_cb[:],
                    w2[:, dt * P:(dt + 1) * P].rearrange(
                        "(i p) d -> p i d", p=P),
                )
                return w2_cb

            # ---------------- software-pipelined chunk loop ----------------
            x_tiles = load_x(0)
            we_sb = router(0)

            for c in range(TC):
                # phase 1: gT[f] = silu(w1.T x) * (v1.T x)   [f32r]
                gts = []
                for f in range(FT):
                    w1_cb = wpool.tile([P, DC, P], F32R, name="w1_cb")
                    nc.sync.dma_start(
                        w1_cb[:],
                        w1[:, f * P:(f + 1) * P].rearrange(
                            "(i p) f -> p i f", p=P),
                    )
                    v1_cb = wpool.tile([P, DC, P], F32R, name="v1_cb")
                    nc.gpsimd.dma_start(
                        v1_cb[:],
                        v1[:, f * P:(f + 1) * P].rearrange(
                            "(i p) f -> p i f", p=P),
                    )
                    ps_h = pmain.tile([P, t_chunk], F32, name="ps_h")
                    for d in range(DC):
                        nc.tensor.matmul(ps_h[:], w1_cb[:, d, :],
                                         x_tiles[d][:],
                                         start=(d == 0), stop=(d == DC - 1))
                    ps_v = pmain.tile([P, t_chunk], F32, name="ps_v")
                    for d in range(DC):
                        nc.tensor.matmul(ps_v[:], v1_cb[:, d, :],
                                         x_tiles[d][:],
                                         start=(d == 0), stop=(d == DC - 1))
                    sl = opool.tile([P, t_chunk], F32, name="sl")
                    nc.scalar.activation(sl[:], ps_h[:], ACTF.Silu)
                    gt = gpool.tile([P, t_chunk], F32R, name="gt")
                    nc.vector.tensor_mul(gt[:], sl[:], ps_v[:])
                    gts.append(gt)

                # expert-weight broadcast for this chunk (inputs long ready)
                we_bc = build_we_bc(we_sb)

                # prefetch first w2 blocks, then next chunk's activations
                w2_pre = [load_w2(0), load_w2(1)]
                if c + 1 < TC:
                    nx_tiles = load_x(c + 1)

                # phase 2: outT[dt] = (sum_f w2[f,dt].T gT[f]) * we
                rs_outs = []
                for part in range(N_RS):
                    rs_in = dram.tile([D // N_RS, t_chunk], F32,
                                      name="rs_in")
                    for k in range(DT // N_RS):
                        dt = part * (DT // N_RS) + k
                        w2_cb = w2_pre[dt] if dt < len(w2_pre) \
                            else load_w2(dt)
                        ps_o = pmain.tile([P, t_chunk], F32, name="ps_o")
                        for f in range(FT):
                            nc.tensor.matmul(ps_o[:], w2_cb[:, f, :],
                                             gts[f][:],
                                             start=(f == 0),
                                             stop=(f == FT - 1))
                        ob = opool.tile([P, t_chunk], F32, name="ob")
                        nc.vector.tensor_mul(ob[:], ps_o[:], we_bc[:])
                        nc.scalar.dma_start(rs_in[k * P:(k + 1) * P, :],
                                            ob[:])
                    rs_out = dramsh.tile([D // N_RS // N_CORES, t_chunk],
                                         F32, name="rs_out")
                    nc.gpsimd.collective_compute(
                        "ReduceScatter",
                        ALU.add,
                        replica_groups=[list(range(N_CORES))],
                        ins=[rs_in[:].opt()],
                        outs=[rs_out[:].opt()],
                    )
                    rs_outs.append(rs_out)

                # next chunk's router (x loaded during phase 2 above)
                if c + 1 < TC:
                    x_tiles = nx_tiles
                    we_sb = router((c + 1) * t_chunk)

                # ship this chunk's shards (waits on RS via tile deps)
                HS = D // N_RS // N_CORES
                for part, rs_out in enumerate(rs_outs):
                    nc.sync.dma_start(
                        out_shards[c, part * HS:(part + 1) * HS, :],
                        rs_out[:])

    nc.finalize()
    return nc


_CACHE = {}
LAST_RESULTS = None


def _get_nc(T, D, F, E, t_chunk):
    key = (T, D, F, E, t_chunk)
    if key not in _CACHE:
        _CACHE[key] = build(*key)
    return _CACHE[key]


def run_moe(hidden_states, router_w, w1, v1, w2, t_chunk=512):
    global LAST_RESULTS
    _install_trace_hook_if_requested()

    B, S, D = hidden_states.shape
    E = router_w.shape[1]
    F = w1.shape[2]
    T = B * S
    DS = D // N_CORES
    TCN = T // t_chunk

    x = np.ascontiguousarray(hidden_states.reshape(T, D).astype(np.float32))
    xT = np.ascontiguousarray(x.T)
    rwc = np.ascontiguousarray(router_w.astype(np.float32))

    nc = _get_nc(T, D, F, E, t_chunk)

    in_maps = []
    for r in range(N_CORES):
        ohr = np.zeros((P, E), dtype=np.float32)
        ohr[:, r] = 1.0
        in_maps.append({
            "xT": xT,
            "w1": np.ascontiguousarray(w1[r].astype(np.float32)),
            "v1": np.ascontiguousarray(v1[r].astype(np.float32)),
            "w2": np.ascontiguousarray(w2[r].astype(np.float32)),
            "rw": rwc,
            "eoh": ohr,
        })

    res = run_bass_kernel_spmd(nc, in_maps, core_ids=list(range(N_CORES)))
    LAST_RESULTS = res

    DTT = D // P
    N_RS = 4 if DTT % 4 == 0 else 2
    HS = D // N_RS // N_CORES
    fullT = np.empty((D, T), dtype=np.float32)
    for r in range(N_CORES):
        sh = res.results[r]["out_shards"]  # [TCN, DS, t_chunk]
        for c in range(TCN):
            cols = slice(c * t_chunk, (c + 1) * t_chunk)
            for h in range(N_RS):
                fullT[h * (D // N_RS) + r * HS:
                      h * (D // N_RS) + (r + 1) * HS, cols] = \
                    sh[c, h * HS:(h + 1) * HS]
    return np.ascontiguousarray(fullT.T).reshape(B, S, D)


def kernel(hidden_states, router_w, w1, v1, w2):
    return run_moe(hidden_states, router_w, w1, v1, w2, t_chunk=512)
